# revision 23
# baseline (speedup 1.0000x reference)
"""Trainium2 Bass kernel for nn_AssociativeMemoryStep (forward-looking retention).

reference semantics:
    q,k,v,o weights = basis @ {q,k,v,o}_coeffs.T          [V, C]
    q/k/v = x @ w                                         [B, T, C]
    scores[t,s] = (q_t . k_s) * decay^(s-t-1) for s>t     (anti-causal)
    retrieved = scores @ v ; out = retrieved @ o_w.T * out_scale

Factored (basis-space) formulation: all four weights share the factor
`basis` [V, 2NB], so with xb = x @ basis [T, 2NB]:
    scores    = (xb @ M) @ xb^T          M  = q_coeffs^T @ k_coeffs  [2NB,2NB]
    out       = (scores_decayed @ xb) @ Wo'
    Wo'       = v_coeffs^T @ o_coeffs @ basis^T * out_scale          [2NB, V]
i.e. retention runs with q := xb@M, k := v := xb. One V-contraction
instead of three (device FLOPs 5.5G -> 3.7G); M and Wo' are tiny
host-precomputed weights (same spirit as the baseline's w = basis@coeffs^T).

Chunkwise-recurrent backward retention, state S_c = sum_{t in chunk c}
decay^t_rel k_t^T v_t (size [2NB,2NB]) from the NEXT chunk only
(decay^256 ~ 4e-6 truncation), L=256 chunks, fully independent.

Sharding: 8 cores = 4 batches x 2 sequence halves of T_loc=2048, each with a
HALO=256 slice of the following positions recomputed locally.

Device layout: xbT ("kT") is [2NB, T_ext]; q~T/qt~T are [2NB, T]; k~ (decay-
scaled) and v copies in [T, 2NB] layout come from PE transposes of xbT.
Projection work for chunk i+1 (transposes + q~) is emitted during chunk i so
the PE never waits on its own PSUM evacuations.
"""

import contextlib

import numpy as np
import ml_dtypes

import concourse.bass as bass
import concourse.mybir as mybir
import concourse.tile as tile
from concourse import bacc
from concourse.bass_utils import run_bass_kernel_spmd

BF16 = ml_dtypes.bfloat16

B, T, V, C = 4, 4096, 1024, 256   # C = 2*N_BASIS = basis channels
N_CORES = 8
T_LOC = 2048          # main positions per core
HALO = 256            # halo positions (state-only; decay^256 ~ 4e-6 truncation)
T_EXT = T_LOC + HALO
L = 256               # retention chunk
PCH = 512             # projection t-chunk
N_MAIN_PCH = T_LOC // PCH  # 4
NCH = T_EXT // L      # 9 L-chunks (8 main + 1 halo)
N_MAIN_CH = T_LOC // L     # 8
KT = V // 128         # 8 v-ktiles
CT = C // 128         # 2 c-tiles

FP32 = mybir.dt.float32
BF = mybir.dt.bfloat16


def build_nc():
    nc = bacc.Bacc("TRN2", target_bir_lowering=False, debug=False,
                   num_devices=N_CORES)

    # all inputs host-pre-tiled to be contiguous per [128, N] DMA block
    xh_d = nc.dram_tensor("xh", [N_MAIN_PCH * 128, KT * PCH], BF, kind="ExternalInput")
    xhh_d = nc.dram_tensor("xhh", [128, KT * HALO], BF, kind="ExternalInput")
    wb_d = nc.dram_tensor("wb", [128, KT * C], BF, kind="ExternalInput")
    mt_d = nc.dram_tensor("mt", [128, CT * C], BF, kind="ExternalInput")
    owT_d = nc.dram_tensor("owT", [128, CT * V], BF, kind="ExternalInput")
    maskT_d = nc.dram_tensor("maskT", [128, CT * L], FP32, kind="ExternalInput")
    crossb_d = nc.dram_tensor("crossb", [128, PCH], FP32, kind="ExternalInput")
    kscale_d = nc.dram_tensor("kscale", [128, 2], FP32, kind="ExternalInput")
    ident_d = nc.dram_tensor("ident", [128, 128], BF, kind="ExternalInput")
    outT_d = nc.dram_tensor("outT", [V, T_LOC], BF, kind="ExternalOutput")

    with tile.TileContext(nc) as tc:
        build_tile(tc, xh_d, xhh_d, wb_d, mt_d, owT_d, maskT_d, crossb_d,
                   kscale_d, ident_d, outT_d)
    nc.compile()
    return nc


def build_tile(tc, xh_d, xhh_d, wb_d, mt_d, owT_d, maskT_d, crossb_d,
               kscale_d, ident_d, outT_d):
    nc = tc.nc

    ctx = contextlib.ExitStack()
    consts = ctx.enter_context(tc.tile_pool(name="consts", bufs=1))
    xpool = ctx.enter_context(tc.tile_pool(name="xpool", bufs=3))
    big = ctx.enter_context(tc.tile_pool(name="big", bufs=1))
    atmp = ctx.enter_context(tc.tile_pool(name="atmp", bufs=4))
    state = ctx.enter_context(tc.tile_pool(name="state", bufs=3))
    ostage = ctx.enter_context(tc.tile_pool(name="ostage", bufs=6))
    psA = ctx.enter_context(tc.tile_pool(name="psA", bufs=2, space="PSUM"))
    psTr = ctx.enter_context(tc.tile_pool(name="psTr", bufs=2, space="PSUM"))
    psB = ctx.enter_context(tc.tile_pool(name="psB", bufs=2, space="PSUM"))
    psO = ctx.enter_context(tc.tile_pool(name="psO", bufs=2, space="PSUM"))

    # ---- constant tiles; DMA order = need order ----
    # wb split into two tiles (kt 0-3 / 4-7) for finer dependency granularity
    wbA_sb = consts.tile([128, 4, C], BF)
    wbB_sb = consts.tile([128, 4, C], BF)
    mt_sb = consts.tile([128, CT, C], BF)
    owT_sb = consts.tile([128, CT * V], BF)
    maskT_sb = consts.tile([128, CT * L], FP32)
    crossb_sb = consts.tile([128, PCH], FP32)
    kscale_sb = consts.tile([128, 2], FP32)
    ident_sb = consts.tile([128, 128], BF)

    def dma_split(out_tile, in_ap, n, engines):
        # split one big contiguous DMA into n pieces so they spread across
        # HWDGE queues (aggregate bandwidth), rotating the issuing engine
        # (each dma_start costs ~650ns serialized on its sequencer)
        if len(out_tile.shape) == 3:
            g = out_tile.shape[1] // n
            b = out_tile.shape[2]
            for i in range(n):
                engines[i % len(engines)].dma_start(
                    out=out_tile[:, i * g:(i + 1) * g, :],
                    in_=in_ap[:, i * g * b:(i + 1) * g * b].rearrange(
                        "p (a b) -> p a b", b=b))
            return
        w = out_tile.shape[-1] // n
        for i in range(n):
            engines[i % len(engines)].dma_start(
                out=out_tile[:, i * w:(i + 1) * w],
                in_=in_ap[:, i * w:(i + 1) * w])

    # ---- persistent activations ----
    kT_sb = big.tile([128, CT, T_EXT], BF)     # xbT: [c-tile, t] main + halo
    qT_sb = big.tile([128, CT, T_LOC], BF)     # q~T = (xb @ M)^T
    qtT_sb = big.tile([128, CT, T_LOC], BF)    # cross-scaled q~T
    ktil_sb = big.tile([128, T_EXT // 128, C], BF)  # decay^t_rel * xb, [t, c]
    v_sb = big.tile([128, T_EXT // 128, C], BF)     # xb in [t, c] layout
    rT_sb = big.tile([128, CT, T_LOC], BF)     # retrieved^T

    # ---- startup DMAs: halo x + basis weights, 128KB pieces, 3 engines ----
    xthA = xpool.tile([128, 4 * HALO], BF, tag="xthA")
    xthB = xpool.tile([128, 4 * HALO], BF, tag="xthB")
    # startup DMA is latency/bandwidth bound (8 cores stream concurrently):
    # 128KB pieces in need-order across the three DMA-capable queues
    nc.sync.dma_start(out=ident_sb, in_=ident_d.ap())   # first: gates warmup
    nc.gpsimd.dma_start(out=kscale_sb, in_=kscale_d.ap())
    start_engines = [nc.sync, nc.gpsimd, nc.scalar]
    pieces = []
    for half in (0, 1):
        wbt = wbA_sb if half == 0 else wbB_sb
        xth = xthA if half == 0 else xthB
        win = wb_d.ap()[:, half * 4 * C:(half + 1) * 4 * C]
        xin = xhh_d.ap()[:, half * 4 * HALO:(half + 1) * 4 * HALO]
        for i in range(2):
            pieces.append((wbt[:, 2 * i:2 * i + 2, :],
                           win[:, i * 2 * C:(i + 1) * 2 * C].rearrange(
                               "p (a b) -> p a b", b=C)))
            w = 2 * HALO
            pieces.append((xth[:, i * w:(i + 1) * w], xin[:, i * w:(i + 1) * w]))
    for i, (dst, src) in enumerate(pieces):
        start_engines[i % 3].dma_start(out=dst, in_=src)
    # remaining small consts on gpsimd (all needed only once qproj/retention
    # start, several us in)
    nc.gpsimd.dma_start(out=mt_sb, in_=mt_d.ap().rearrange("p (a b) -> p a b", b=C))
    nc.gpsimd.dma_start(out=crossb_sb, in_=crossb_d.ap())
    nc.gpsimd.dma_start(out=maskT_sb, in_=maskT_d.ap())

    # ---- PE clock warmup: the HAM throttle needs ~3.4us of sustained PE
    # activity to release half-rate clocking; burn the initial DMA wait on
    # dummy transposes of ident so the real matmuls start at full clock
    for i in range(26):
        if i % 2 == 0:
            wps = psTr.tile([128, 128], BF, tag="pst", name="warm")
        else:
            wps = psB.tile([128, 128], BF, tag="ps", name="warm")
        nc.tensor.transpose(wps, ident_sb, ident_sb)

    # ---- halo xbT (kt-major so each DMA piece gates only its own matmuls) ----
    hps0 = psA.tile([128, HALO], FP32, tag="ps")
    hps1 = psA.tile([128, HALO], FP32, tag="ps")
    hps = [hps0, hps1]
    for kt in range(KT):
        xth = xthA if kt < 4 else xthB
        wbt = wbA_sb if kt < 4 else wbB_sb
        for ct in range(CT):
            nc.tensor.matmul(
                hps[ct], lhsT=wbt[:, kt % 4, ct * 128:(ct + 1) * 128],
                rhs=xth[:, (kt % 4) * HALO:(kt % 4 + 1) * HALO],
                start=(kt == 0), stop=(kt == KT - 1))
    nc.scalar.copy(kT_sb[:, 0, T_LOC:T_EXT], hps[0])
    nc.vector.tensor_copy(kT_sb[:, 1, T_LOC:T_EXT], hps[1])

    def transposes(t0, nt):
        # k~ and v (both [t, c] layout) via PE transpose of xbT; ct0/ct1 go
        # into one [128, C] psum so the scale/copy evacuations are 256-wide.
        # v copies are emitted later (v_copies) so the ACT queue serves the
        # urgent qT evacuations first.
        psts = []
        for tb in range(nt):
            tt = t0 // 128 + tb
            if tb % 2 == 0:
                pst = psTr.tile([128, C], BF, tag="pst")
            else:
                pst = psB.tile([128, C], BF, tag="ps")
            for ct in range(CT):
                nc.tensor.transpose(
                    pst[:, ct * 128:(ct + 1) * 128],
                    kT_sb[:, ct, t0 + tb * 128:t0 + (tb + 1) * 128], ident_sb)
            nc.vector.tensor_scalar_mul(
                ktil_sb[:, tt, :], pst, kscale_sb[:, (tt % 2):(tt % 2) + 1])
            psts.append((pst, tt))
        return psts

    def v_copies(psts):
        for pst, tt in psts:
            nc.scalar.copy(v_sb[:, tt, :], pst)

    def qproj(pch):
        # q~T = M^T @ xbT (contract over c'). Uses psO (idle until retention)
        # so the slot handoff never blocks the next chunk's xbT matmuls; the
        # psum has a SINGLE reader (Tile serializes same-bank readers), with
        # the cross-scaled copy computed from SBUF later (qtT_muls).
        t0 = pch * PCH
        for ct in range(CT):
            qps = psO.tile([128, PCH], FP32, tag="po", name="qps")
            for cp in range(CT):
                nc.tensor.matmul(
                    qps, lhsT=mt_sb[:, cp, ct * 128:(ct + 1) * 128],
                    rhs=kT_sb[:, cp, t0:t0 + PCH],
                    start=(cp == 0), stop=(cp == CT - 1))
            nc.scalar.copy(qT_sb[:, ct, t0:t0 + PCH], qps)
        for ct in range(CT):
            nc.vector.tensor_mul(qtT_sb[:, ct, t0:t0 + PCH],
                                 qT_sb[:, ct, t0:t0 + PCH], crossb_sb)

    # ---- main projections, reverse t-chunk order, 1-chunk pipelined ----
    for idx, pch in enumerate(range(N_MAIN_PCH - 1, -1, -1)):
        t0 = pch * PCH
        xt = xpool.tile([128, KT * PCH], BF, tag="xt")
        dma_split(xt, xh_d.ap()[pch * 128:(pch + 1) * 128, :], 8,
                  [nc.sync, nc.gpsimd, nc.scalar] if idx == 0
                  else [nc.sync, nc.gpsimd])
        if idx == 1:
            dma_split(owT_sb, owT_d.ap(), 4, [nc.sync, nc.gpsimd])
        # pipelined: previous chunk's transposes + q~ run while this chunk's
        # x streams in (their inputs were copied out a full chunk ago)
        if idx == 0:
            psts = transposes(T_LOC, HALO // 128)
        else:
            psts = transposes((pch + 1) * PCH, PCH // 128)
            qproj(pch + 1)
        pcs = [psA.tile([128, PCH], FP32, tag="ps", name="xps")
               for _ in range(CT)]
        for kt in range(KT):
            wbt = wbA_sb if kt < 4 else wbB_sb
            for ct in range(CT):
                nc.tensor.matmul(
                    pcs[ct], lhsT=wbt[:, kt % 4, ct * 128:(ct + 1) * 128],
                    rhs=xt[:, kt * PCH:(kt + 1) * PCH],
                    start=(kt == 0), stop=(kt == KT - 1))
        # queue order matters: kT evacs free the psA slots the NEXT chunk's
        # xbT needs, so they go before the deferrable qtT muls / v copies
        nc.scalar.copy(kT_sb[:, 0, t0:t0 + PCH], pcs[0])
        nc.vector.tensor_copy(kT_sb[:, 1, t0:t0 + PCH], pcs[1])
        v_copies(psts)
    psts = transposes(0, PCH // 128)
    qproj(0)
    v_copies(psts)

    # ---- phase 2+3 interleaved: backward retention chunks + output proj ----
    S_cur = None

    def retention_chunk(c):
        nonlocal S_cur
        is_main = c < N_MAIN_CH
        c0 = c * L
        tt0 = c0 // 128  # first t-tile of this chunk (2 per chunk)
        atm = []
        rps = []
        if is_main:
            # AT[j, i] = sum_c' k[j,c'] q[i,c'] ; masked -> atm (bf16)
            for jt in range(2):
                ps = psA.tile([128, L], FP32, tag="ps")
                for ct in range(CT):
                    nc.tensor.matmul(
                        ps, lhsT=kT_sb[:, ct, c0 + jt * 128:c0 + (jt + 1) * 128],
                        rhs=qT_sb[:, ct, c0:c0 + L],
                        start=(ct == 0), stop=(ct == CT - 1))
                am = atmp.tile([128, L], BF, tag="atm")
                nc.vector.tensor_mul(am, ps, maskT_sb[:, jt * L:(jt + 1) * L])
                atm.append(am)
            # cross part first: S^T @ q~T needs no masks, so PE keeps running
            # while DVE applies them
            for ct in range(CT):
                ps = psB.tile([128, L], FP32, tag="ps")
                for st in range(CT):
                    nc.tensor.matmul(
                        ps, lhsT=S_cur[:, st, ct * 128:(ct + 1) * 128],
                        rhs=qtT_sb[:, st, c0:c0 + L],
                        start=(st == 0), stop=False)
                rps.append(ps)
        # state: S = k~^T v of THIS chunk only (decay^L * older state ~ 4e-6,
        # numerically negligible -> no recursion, chunks fully independent).
        if c > 0:
            S_new = state.tile([128, CT, C], BF, tag="S")
            for st in range(CT):
                ps = psTr.tile([128, C], FP32, tag="pst")
                for jt in range(2):
                    nc.tensor.matmul(
                        ps, lhsT=ktil_sb[:, tt0 + jt, st * 128:(st + 1) * 128],
                        rhs=v_sb[:, tt0 + jt, :],
                        start=(jt == 0), stop=(jt == 1))
                nc.scalar.copy(S_new[:, st, :], ps)
            S_cur = S_new
        if is_main:
            # intra part: v^T @ atm accumulated into the same rT psum
            for ct in range(CT):
                for jt in range(2):
                    nc.tensor.matmul(
                        rps[ct], lhsT=v_sb[:, tt0 + jt, ct * 128:(ct + 1) * 128],
                        rhs=atm[jt], start=False, stop=(jt == 1))
                nc.vector.tensor_copy(rT_sb[:, ct, c0:c0 + L], rps[ct])

    def outproj(t0, width, pools, final=False):
        # out[v, t0:t0+width]: owT^T @ rT. Output staged 4 v-tiles at a time
        # into one tile -> ONE wide HWDGE store per group (2 per call); no
        # SWDGE stores, so the kernel-end gpsimd DRAIN never waits on them.
        gsz = 4
        ogs = [ostage.tile([128, gsz, width], BF, tag="og", name="og",
                           padded_shape=[128, gsz, PCH])
               for _ in range(KT // gsz)]
        for vt in range(KT):
            pool, tag = pools[vt % len(pools)]
            ps = pool.tile([128, width], FP32, tag=tag, name="ops")
            for ct in range(CT):
                nc.tensor.matmul(
                    ps,
                    lhsT=owT_sb[:, ct * V + vt * 128:ct * V + (vt + 1) * 128],
                    rhs=rT_sb[:, ct, t0:t0 + width],
                    start=(ct == 0), stop=(ct == CT - 1))
            # alternate evac engines so consecutive psums drain concurrently
            cp = nc.vector.tensor_copy if vt % 2 == 0 else nc.scalar.copy
            cp(ogs[vt // gsz][:, vt % gsz, :], ps)
            if vt % gsz == gsz - 1:
                g0 = vt - gsz + 1
                eng = nc.scalar if (final and (vt // gsz) % 2 == 1) else nc.sync
                eng.dma_start(
                    out=outT_d.ap()[g0 * 128:(vt + 1) * 128,
                                    t0:t0 + width].rearrange(
                                        "(a p) t -> p a t", p=128),
                    in_=ogs[vt // gsz])

    # emit outproj one chunk after its rT inputs complete, so the rT
    # evacuations (DVE) have slack before PE consumes them; the last two
    # chunks project at L-granularity so the post-retention tail is short
    for c in range(NCH - 1, -1, -1):
        retention_chunk(c)
        if c in (5, 3):
            outproj((c // 2 + 1) * PCH, PCH, pools=[(psO, "po"), (psB, "ps")])
        elif c == 1:
            outproj(PCH, PCH, pools=[(psO, "po"), (psB, "ps")])
            outproj(L, L, pools=[(psO, "po"), (psB, "ps")])
    outproj(0, L, final=True,
            pools=[(psO, "po"), (psTr, "pst"), (psA, "ps")])

    ctx.close()


# ---------------- host side ----------------

_NC_CACHE = None


def _get_nc():
    global _NC_CACHE
    if _NC_CACHE is None:
        _NC_CACHE = build_nc()
    return _NC_CACHE


def _tile128(a, inner):
    """[G*128, inner] -> [128, G*inner] with block g at columns [g*inner,...)."""
    g = a.shape[0] // 128
    return np.ascontiguousarray(
        a.reshape(g, 128, inner).transpose(1, 0, 2).reshape(128, g * inner))


def _prep_in_maps(inputs):
    x = np.asarray(inputs["x"], np.float32)
    basis = np.asarray(inputs["basis"], np.float32)
    decay = float(1.0 / (1.0 + np.exp(-np.float64(inputs["decay_logit"]))))
    out_scale = float(np.float32(inputs["out_scale"]))

    qc = np.asarray(inputs["q_coeffs"], np.float32)
    kc = np.asarray(inputs["k_coeffs"], np.float32)
    vc = np.asarray(inputs["v_coeffs"], np.float32)
    oc = np.asarray(inputs["o_coeffs"], np.float32)

    wb = _tile128(basis.astype(BF16), C)                       # [128, KT*C]
    mt = _tile128((qc.T @ kc).astype(BF16), C)                 # [128, CT*C]
    owT = _tile128(((vc.T @ oc) @ basis.T * out_scale).astype(BF16), V)

    i = np.arange(L)
    jj, ii = np.meshgrid(i, i, indexing="ij")
    maskT = np.where(jj > ii, decay ** np.maximum(jj - ii - 1, 0), 0.0).astype(np.float32)
    maskT = _tile128(maskT, L)
    cross = (decay ** (L - 1 - i)).astype(np.float32)
    crossb = np.ascontiguousarray(
        np.broadcast_to(np.tile(cross, PCH // L)[None, :], (128, PCH)), np.float32)
    ksc = decay ** np.arange(2 * 128, dtype=np.float64)
    kscale = np.stack([ksc[:128], ksc[128:]], axis=1).astype(np.float32)
    ident = np.eye(128, dtype=np.float32).astype(BF16)

    in_maps = []
    for core in range(N_CORES):
        b, h = divmod(core, 2)
        t0 = h * T_LOC
        te = min(t0 + T_EXT, T)
        xT = np.zeros((V, T_EXT), dtype=BF16)
        xT[:, :te - t0] = x[b, t0:te].T.astype(BF16)
        # pre-tile main: [pch][p][kt][t] contiguous; halo: [p][kt][t]
        xh = np.ascontiguousarray(
            xT[:, :T_LOC].reshape(KT, 128, N_MAIN_PCH, PCH).transpose(2, 1, 0, 3)
        ).reshape(N_MAIN_PCH * 128, KT * PCH)
        xhh = np.ascontiguousarray(
            xT[:, T_LOC:].reshape(KT, 128, HALO).transpose(1, 0, 2)
        ).reshape(128, KT * HALO)
        in_maps.append({
            "xh": xh, "xhh": xhh, "wb": wb, "mt": mt, "owT": owT,
            "maskT": maskT, "crossb": crossb, "kscale": kscale,
            "ident": ident,
        })
    return in_maps


def _ensure_ntff_hook():
    """The agent image's antenv package lacks axon_hooks; shim it so
    run_bass_kernel_spmd(trace=True) can register the NTFF profile hook."""
    try:
        from antenv.axon_hooks import get_axon_ntff_profile_hook  # noqa: F401
        return
    except ImportError:
        pass
    import sys
    import types
    import antenv
    mod = types.ModuleType("antenv.axon_hooks")
    _state = {"hook": None}
    mod.set_axon_ntff_profile_hook = lambda h: _state.__setitem__("hook", h)
    mod.get_axon_ntff_profile_hook = lambda: _state["hook"]
    sys.modules["antenv.axon_hooks"] = mod
    antenv.axon_hooks = mod
    from trn_agent_boot.trn_boot import _ntff_profile_via_ctypes
    mod.set_axon_ntff_profile_hook(
        _ntff_profile_via_ctypes("/opt/axon/libaxon_pjrt.so"))


def run(inputs, trace=False):
    """Returns (out [B,T,V] float32, BassKernelResults)."""
    if trace:
        _ensure_ntff_hook()
    in_maps = _prep_in_maps(inputs)
    nc = _get_nc()
    res = run_bass_kernel_spmd(nc, in_maps, core_ids=list(range(N_CORES)),
                               trace=trace)
    out = np.zeros((B, T, V), np.float32)
    for core in range(N_CORES):
        b, h = divmod(core, 2)
        outT = np.asarray(res.results[core]["outT"]).astype(np.float32)
        out[b, h * T_LOC:(h + 1) * T_LOC] = outT.T
    return out, res


def kernel(**inputs):
    out, _ = run(inputs, trace=False)
    return out


# revision 27
# speedup vs baseline: 1.0257x; 1.0257x over previous
"""Trainium2 Bass kernel for nn_AssociativeMemoryStep (forward-looking retention).

reference semantics:
    q,k,v,o weights = basis @ {q,k,v,o}_coeffs.T          [V, C]
    q/k/v = x @ w                                         [B, T, C]
    scores[t,s] = (q_t . k_s) * decay^(s-t-1) for s>t     (anti-causal)
    retrieved = scores @ v ; out = retrieved @ o_w.T * out_scale

Factored (basis-space) formulation: all four weights share the factor
`basis` [V, 2NB], so with xb = x @ basis [T, 2NB]:
    scores    = (xb @ M) @ xb^T          M  = q_coeffs^T @ k_coeffs  [2NB,2NB]
    out       = (scores_decayed @ xb) @ Wo'
    Wo'       = v_coeffs^T @ o_coeffs @ basis^T * out_scale          [2NB, V]
i.e. retention runs with q := xb@M, k := v := xb. One V-contraction
instead of three (device FLOPs 5.5G -> 3.7G); M and Wo' are tiny
host-precomputed weights (same spirit as the baseline's w = basis@coeffs^T).

Chunkwise-recurrent backward retention, state S_c = sum_{t in chunk c}
decay^t_rel k_t^T v_t (size [2NB,2NB]) from the NEXT chunk only
(decay^256 ~ 4e-6 truncation), L=256 chunks, fully independent.

Sharding: 8 cores = 4 batches x 2 sequence halves of T_loc=2048, each with a
HALO=256 slice of the following positions recomputed locally.

Device layout: xbT ("kT") is [2NB, T_ext]; q~T/qt~T are [2NB, T]; k~ (decay-
scaled) and v copies in [T, 2NB] layout come from PE transposes of xbT.
Projection work for chunk i+1 (transposes + q~) is emitted during chunk i so
the PE never waits on its own PSUM evacuations.
"""

import contextlib

import numpy as np
import ml_dtypes

import concourse.bass as bass
import concourse.mybir as mybir
import concourse.tile as tile
from concourse import bacc
from concourse.bass_utils import run_bass_kernel_spmd

BF16 = ml_dtypes.bfloat16

B, T, V, C = 4, 4096, 1024, 256   # C = 2*N_BASIS = basis channels
N_CORES = 8
T_LOC = 2048          # main positions per core
HALO = 256            # halo positions (state-only; decay^256 ~ 4e-6 truncation)
T_EXT = T_LOC + HALO
L = 256               # retention chunk
PCH = 512             # projection t-chunk
N_MAIN_PCH = T_LOC // PCH  # 4
NCH = T_EXT // L      # 9 L-chunks (8 main + 1 halo)
N_MAIN_CH = T_LOC // L     # 8
KT = V // 128         # 8 v-ktiles
CT = C // 128         # 2 c-tiles

FP32 = mybir.dt.float32
BF = mybir.dt.bfloat16


def build_nc():
    nc = bacc.Bacc("TRN2", target_bir_lowering=False, debug=False,
                   num_devices=N_CORES)

    # all inputs host-pre-tiled to be contiguous per [128, N] DMA block
    xh_d = nc.dram_tensor("xh", [N_MAIN_PCH * 128, KT * PCH], BF, kind="ExternalInput")
    xhh_d = nc.dram_tensor("xhh", [128, KT * HALO], BF, kind="ExternalInput")
    wb_d = nc.dram_tensor("wb", [128, KT * C], BF, kind="ExternalInput")
    mt_d = nc.dram_tensor("mt", [128, CT * C], BF, kind="ExternalInput")
    owT_d = nc.dram_tensor("owT", [128, CT * V], BF, kind="ExternalInput")
    maskT_d = nc.dram_tensor("maskT", [128, CT * L], FP32, kind="ExternalInput")
    crossb_d = nc.dram_tensor("crossb", [128, PCH], FP32, kind="ExternalInput")
    kscale_d = nc.dram_tensor("kscale", [128, 2], FP32, kind="ExternalInput")
    ident_d = nc.dram_tensor("ident", [128, 128], BF, kind="ExternalInput")
    outT_d = nc.dram_tensor("outT", [V, T_LOC], BF, kind="ExternalOutput")

    with tile.TileContext(nc) as tc:
        build_tile(tc, xh_d, xhh_d, wb_d, mt_d, owT_d, maskT_d, crossb_d,
                   kscale_d, ident_d, outT_d)
    nc.compile()
    return nc


def build_tile(tc, xh_d, xhh_d, wb_d, mt_d, owT_d, maskT_d, crossb_d,
               kscale_d, ident_d, outT_d):
    nc = tc.nc

    ctx = contextlib.ExitStack()
    consts = ctx.enter_context(tc.tile_pool(name="consts", bufs=1))
    xpool = ctx.enter_context(tc.tile_pool(name="xpool", bufs=3))
    big = ctx.enter_context(tc.tile_pool(name="big", bufs=1))
    atmp = ctx.enter_context(tc.tile_pool(name="atmp", bufs=4))
    state = ctx.enter_context(tc.tile_pool(name="state", bufs=3))
    ostage = ctx.enter_context(tc.tile_pool(name="ostage", bufs=6))
    psA = ctx.enter_context(tc.tile_pool(name="psA", bufs=2, space="PSUM"))
    psTr = ctx.enter_context(tc.tile_pool(name="psTr", bufs=2, space="PSUM"))
    psB = ctx.enter_context(tc.tile_pool(name="psB", bufs=2, space="PSUM"))
    psO = ctx.enter_context(tc.tile_pool(name="psO", bufs=2, space="PSUM"))

    # ---- constant tiles; DMA order = need order ----
    # wb split into two tiles (kt 0-3 / 4-7) for finer dependency granularity
    wbA_sb = consts.tile([128, 4, C], BF)
    wbB_sb = consts.tile([128, 4, C], BF)
    mt_sb = consts.tile([128, CT, C], BF)
    owT_sb = consts.tile([128, CT * V], BF)
    maskT_sb = consts.tile([128, CT * L], FP32)
    crossb_sb = consts.tile([128, PCH], FP32)
    kscale_sb = consts.tile([128, 2], FP32)
    ident_sb = consts.tile([128, 128], BF)

    def dma_split(out_tile, in_ap, n, engines):
        # split one big contiguous DMA into n pieces so they spread across
        # HWDGE queues (aggregate bandwidth), rotating the issuing engine
        # (each dma_start costs ~650ns serialized on its sequencer)
        if len(out_tile.shape) == 3:
            g = out_tile.shape[1] // n
            b = out_tile.shape[2]
            for i in range(n):
                engines[i % len(engines)].dma_start(
                    out=out_tile[:, i * g:(i + 1) * g, :],
                    in_=in_ap[:, i * g * b:(i + 1) * g * b].rearrange(
                        "p (a b) -> p a b", b=b))
            return
        w = out_tile.shape[-1] // n
        for i in range(n):
            engines[i % len(engines)].dma_start(
                out=out_tile[:, i * w:(i + 1) * w],
                in_=in_ap[:, i * w:(i + 1) * w])

    # ---- persistent activations ----
    kT_sb = big.tile([128, CT, T_EXT], BF)     # xbT: [c-tile, t] main + halo
    qT_sb = big.tile([128, CT, T_LOC], BF)     # q~T = (xb @ M)^T
    qtT_sb = big.tile([128, CT, T_LOC], BF)    # cross-scaled q~T
    ktil_sb = big.tile([128, T_EXT // 128, C], BF)  # decay^t_rel * xb, [t, c]
    v_sb = big.tile([128, T_EXT // 128, C], BF)     # xb in [t, c] layout
    rT_sb = big.tile([128, CT, T_LOC], BF)     # retrieved^T

    # ---- startup DMAs: halo x + basis weights, 128KB pieces, 3 engines ----
    xthA = xpool.tile([128, 4 * HALO], BF, tag="xthA")
    xthB = xpool.tile([128, 4 * HALO], BF, tag="xthB")
    # startup DMA is latency/bandwidth bound (8 cores stream concurrently):
    # 128KB pieces in need-order across the three DMA-capable queues
    nc.sync.dma_start(out=ident_sb, in_=ident_d.ap())   # first: gates warmup
    nc.gpsimd.dma_start(out=kscale_sb, in_=kscale_d.ap())
    start_engines = [nc.sync, nc.gpsimd, nc.scalar]
    pieces = []
    for half in (0, 1):
        wbt = wbA_sb if half == 0 else wbB_sb
        xth = xthA if half == 0 else xthB
        win = wb_d.ap()[:, half * 4 * C:(half + 1) * 4 * C]
        xin = xhh_d.ap()[:, half * 4 * HALO:(half + 1) * 4 * HALO]
        for i in range(2):
            pieces.append((wbt[:, 2 * i:2 * i + 2, :],
                           win[:, i * 2 * C:(i + 1) * 2 * C].rearrange(
                               "p (a b) -> p a b", b=C)))
            w = 2 * HALO
            pieces.append((xth[:, i * w:(i + 1) * w], xin[:, i * w:(i + 1) * w]))
    for i, (dst, src) in enumerate(pieces):
        start_engines[i % 3].dma_start(out=dst, in_=src)
    # remaining small consts on gpsimd (all needed only once qproj/retention
    # start, several us in)
    nc.gpsimd.dma_start(out=mt_sb, in_=mt_d.ap().rearrange("p (a b) -> p a b", b=C))
    nc.gpsimd.dma_start(out=crossb_sb, in_=crossb_d.ap())
    nc.gpsimd.dma_start(out=maskT_sb, in_=maskT_d.ap())

    # ---- PE clock warmup: the HAM throttle needs ~3.4us of sustained PE
    # activity to release half-rate clocking; burn the initial DMA wait on
    # dummy transposes of ident so the real matmuls start at full clock
    for i in range(16):
        if i % 2 == 0:
            wps = psTr.tile([128, 128], BF, tag="pst", name="warm")
        else:
            wps = psB.tile([128, 128], BF, tag="ps", name="warm")
        nc.tensor.transpose(wps, ident_sb, ident_sb)
    # scratch psum for fillers interleaved with the DMA-paced halo matmuls
    # (slot is recycled by later real allocs; all filler writes precede them)
    warm_ps = psB.tile([128, 128], BF, tag="ps", name="warm_ps")

    # ---- halo xbT (kt-major so each DMA piece gates only its own matmuls) ----
    hps0 = psA.tile([128, HALO], FP32, tag="ps")
    hps1 = psA.tile([128, HALO], FP32, tag="ps")
    hps = [hps0, hps1]
    for kt in range(KT):
        xth = xthA if kt < 4 else xthB
        wbt = wbA_sb if kt < 4 else wbB_sb
        for ct in range(CT):
            nc.tensor.matmul(
                hps[ct], lhsT=wbt[:, kt % 4, ct * 128:(ct + 1) * 128],
                rhs=xth[:, (kt % 4) * HALO:(kt % 4 + 1) * HALO],
                start=(kt == 0), stop=(kt == KT - 1))
        # fillers run whenever the next kt's data hasn't landed yet, keeping
        # the PE activity monitor hot through the DMA-paced halo phase
        nc.tensor.transpose(warm_ps, ident_sb, ident_sb)
        nc.tensor.transpose(warm_ps, ident_sb, ident_sb)
    nc.scalar.copy(kT_sb[:, 0, T_LOC:T_EXT], hps[0])
    nc.vector.tensor_copy(kT_sb[:, 1, T_LOC:T_EXT], hps[1])

    def transposes(t0, nt):
        # k~ and v (both [t, c] layout) via PE transpose of xbT; ct0/ct1 go
        # into one [128, C] psum so the scale/copy evacuations are 256-wide.
        # v copies are emitted later (v_copies) so the ACT queue serves the
        # urgent qT evacuations first.
        psts = []
        for tb in range(nt):
            tt = t0 // 128 + tb
            if tb % 2 == 0:
                pst = psTr.tile([128, C], BF, tag="pst")
            else:
                pst = psB.tile([128, C], BF, tag="ps")
            for ct in range(CT):
                nc.tensor.transpose(
                    pst[:, ct * 128:(ct + 1) * 128],
                    kT_sb[:, ct, t0 + tb * 128:t0 + (tb + 1) * 128], ident_sb)
            nc.vector.tensor_scalar_mul(
                ktil_sb[:, tt, :], pst, kscale_sb[:, (tt % 2):(tt % 2) + 1])
            psts.append((pst, tt))
        return psts

    def v_copies(psts):
        for pst, tt in psts:
            nc.scalar.copy(v_sb[:, tt, :], pst)

    def qproj(pch):
        # q~T = M^T @ xbT (contract over c'). Uses psO (idle until retention)
        # so the slot handoff never blocks the next chunk's xbT matmuls; the
        # psum has a SINGLE reader (Tile serializes same-bank readers), with
        # the cross-scaled copy computed from SBUF later (qtT_muls).
        t0 = pch * PCH
        for ct in range(CT):
            qps = psO.tile([128, PCH], FP32, tag="po", name="qps")
            for cp in range(CT):
                nc.tensor.matmul(
                    qps, lhsT=mt_sb[:, cp, ct * 128:(ct + 1) * 128],
                    rhs=kT_sb[:, cp, t0:t0 + PCH],
                    start=(cp == 0), stop=(cp == CT - 1))
            nc.scalar.copy(qT_sb[:, ct, t0:t0 + PCH], qps)

    def qtT_muls(pch):
        t0 = pch * PCH
        for ct in range(CT):
            nc.vector.tensor_mul(qtT_sb[:, ct, t0:t0 + PCH],
                                 qT_sb[:, ct, t0:t0 + PCH], crossb_sb)

    # ---- main projections, reverse t-chunk order, 1-chunk pipelined ----
    for idx, pch in enumerate(range(N_MAIN_PCH - 1, -1, -1)):
        t0 = pch * PCH
        xt = xpool.tile([128, KT * PCH], BF, tag="xt")
        dma_split(xt, xh_d.ap()[pch * 128:(pch + 1) * 128, :], 8,
                  [nc.sync, nc.gpsimd, nc.scalar] if idx == 0
                  else [nc.sync, nc.gpsimd])
        if idx == 1:
            dma_split(owT_sb, owT_d.ap(), 4, [nc.sync, nc.gpsimd])
        # pipelined: previous chunk's transposes + q~ run while this chunk's
        # x streams in (their inputs were copied out a full chunk ago)
        if idx == 0:
            psts = transposes(T_LOC, HALO // 128)
        else:
            psts = transposes((pch + 1) * PCH, PCH // 128)
            qproj(pch + 1)
        pcs = [psA.tile([128, PCH], FP32, tag="ps", name="xps")
               for _ in range(CT)]
        for kt in range(KT):
            wbt = wbA_sb if kt < 4 else wbB_sb
            for ct in range(CT):
                nc.tensor.matmul(
                    pcs[ct], lhsT=wbt[:, kt % 4, ct * 128:(ct + 1) * 128],
                    rhs=xt[:, kt * PCH:(kt + 1) * PCH],
                    start=(kt == 0), stop=(kt == KT - 1))
        # queue order matters: kT evacs free the psA slots the NEXT chunk's
        # xbT needs, so they go before the deferrable qtT muls / v copies
        # queue order matters: the kT evacs free the psA slots the NEXT
        # chunk's xbT needs, so they precede the deferrable qtT muls/v copies
        nc.scalar.copy(kT_sb[:, 0, t0:t0 + PCH], pcs[0])
        nc.vector.tensor_copy(kT_sb[:, 1, t0:t0 + PCH], pcs[1])
        if idx != 0:
            qtT_muls(pch + 1)
        v_copies(psts)
    psts = transposes(0, PCH // 128)
    qproj(0)
    qtT_muls(0)
    v_copies(psts)

    # ---- phase 2+3 interleaved: backward retention chunks + output proj ----
    S_cur = None

    def retention_chunk(c):
        nonlocal S_cur
        is_main = c < N_MAIN_CH
        c0 = c * L
        tt0 = c0 // 128  # first t-tile of this chunk (2 per chunk)
        atm = []
        rps = []
        if is_main:
            # AT[j, i] = sum_c' k[j,c'] q[i,c'] ; masked -> atm (bf16)
            for jt in range(2):
                ps = psA.tile([128, L], FP32, tag="ps")
                for ct in range(CT):
                    nc.tensor.matmul(
                        ps, lhsT=kT_sb[:, ct, c0 + jt * 128:c0 + (jt + 1) * 128],
                        rhs=qT_sb[:, ct, c0:c0 + L],
                        start=(ct == 0), stop=(ct == CT - 1))
                am = atmp.tile([128, L], BF, tag="atm")
                nc.vector.tensor_mul(am, ps, maskT_sb[:, jt * L:(jt + 1) * L])
                atm.append(am)
            # cross part first: S^T @ q~T needs no masks, so PE keeps running
            # while DVE applies them
            for ct in range(CT):
                ps = psB.tile([128, L], FP32, tag="ps")
                for st in range(CT):
                    nc.tensor.matmul(
                        ps, lhsT=S_cur[:, st, ct * 128:(ct + 1) * 128],
                        rhs=qtT_sb[:, st, c0:c0 + L],
                        start=(st == 0), stop=False)
                rps.append(ps)
        # state: S = k~^T v of THIS chunk only (decay^L * older state ~ 4e-6,
        # numerically negligible -> no recursion, chunks fully independent).
        if c > 0:
            S_new = state.tile([128, CT, C], BF, tag="S")
            for st in range(CT):
                ps = psTr.tile([128, C], FP32, tag="pst")
                for jt in range(2):
                    nc.tensor.matmul(
                        ps, lhsT=ktil_sb[:, tt0 + jt, st * 128:(st + 1) * 128],
                        rhs=v_sb[:, tt0 + jt, :],
                        start=(jt == 0), stop=(jt == 1))
                nc.scalar.copy(S_new[:, st, :], ps)
            S_cur = S_new
        if is_main:
            # intra part: v^T @ atm accumulated into the same rT psum
            for ct in range(CT):
                for jt in range(2):
                    nc.tensor.matmul(
                        rps[ct], lhsT=v_sb[:, tt0 + jt, ct * 128:(ct + 1) * 128],
                        rhs=atm[jt], start=False, stop=(jt == 1))
                nc.vector.tensor_copy(rT_sb[:, ct, c0:c0 + L], rps[ct])

    def outproj(t0, width, pools, final=False):
        # out[v, t0:t0+width]: owT^T @ rT. Output staged 4 v-tiles at a time
        # into one tile -> ONE wide HWDGE store per group (2 per call); no
        # SWDGE stores, so the kernel-end gpsimd DRAIN never waits on them.
        gsz = 4
        ogs = [ostage.tile([128, gsz, width], BF, tag="og", name="og",
                           padded_shape=[128, gsz, PCH])
               for _ in range(KT // gsz)]
        for vt in range(KT):
            pool, tag = pools[vt % len(pools)]
            ps = pool.tile([128, width], FP32, tag=tag, name="ops")
            for ct in range(CT):
                nc.tensor.matmul(
                    ps,
                    lhsT=owT_sb[:, ct * V + vt * 128:ct * V + (vt + 1) * 128],
                    rhs=rT_sb[:, ct, t0:t0 + width],
                    start=(ct == 0), stop=(ct == CT - 1))
            # alternate evac engines so consecutive psums drain concurrently
            cp = nc.vector.tensor_copy if vt % 2 == 0 else nc.scalar.copy
            cp(ogs[vt // gsz][:, vt % gsz, :], ps)
            if vt % gsz == gsz - 1:
                g0 = vt - gsz + 1
                eng = nc.scalar if (final and (vt // gsz) % 2 == 1) else nc.sync
                eng.dma_start(
                    out=outT_d.ap()[g0 * 128:(vt + 1) * 128,
                                    t0:t0 + width].rearrange(
                                        "(a p) t -> p a t", p=128),
                    in_=ogs[vt // gsz])

    # emit outproj one chunk after its rT inputs complete, so the rT
    # evacuations (DVE) have slack before PE consumes them; the last two
    # chunks project at L-granularity so the post-retention tail is short
    for c in range(NCH - 1, -1, -1):
        retention_chunk(c)
        if c in (5, 3):
            outproj((c // 2 + 1) * PCH, PCH, pools=[(psO, "po"), (psB, "ps")])
        elif c == 1:
            outproj(PCH, PCH, pools=[(psO, "po"), (psB, "ps")])
            outproj(L, L, pools=[(psO, "po"), (psB, "ps")])
    outproj(0, L, final=True,
            pools=[(psO, "po"), (psTr, "pst"), (psA, "ps")])

    ctx.close()


# ---------------- host side ----------------

_NC_CACHE = None


def _get_nc():
    global _NC_CACHE
    if _NC_CACHE is None:
        _NC_CACHE = build_nc()
    return _NC_CACHE


def _tile128(a, inner):
    """[G*128, inner] -> [128, G*inner] with block g at columns [g*inner,...)."""
    g = a.shape[0] // 128
    return np.ascontiguousarray(
        a.reshape(g, 128, inner).transpose(1, 0, 2).reshape(128, g * inner))


def _prep_in_maps(inputs):
    x = np.asarray(inputs["x"], np.float32)
    basis = np.asarray(inputs["basis"], np.float32)
    decay = float(1.0 / (1.0 + np.exp(-np.float64(inputs["decay_logit"]))))
    out_scale = float(np.float32(inputs["out_scale"]))

    qc = np.asarray(inputs["q_coeffs"], np.float32)
    kc = np.asarray(inputs["k_coeffs"], np.float32)
    vc = np.asarray(inputs["v_coeffs"], np.float32)
    oc = np.asarray(inputs["o_coeffs"], np.float32)

    wb = _tile128(basis.astype(BF16), C)                       # [128, KT*C]
    mt = _tile128((qc.T @ kc).astype(BF16), C)                 # [128, CT*C]
    owT = _tile128(((vc.T @ oc) @ basis.T * out_scale).astype(BF16), V)

    i = np.arange(L)
    jj, ii = np.meshgrid(i, i, indexing="ij")
    maskT = np.where(jj > ii, decay ** np.maximum(jj - ii - 1, 0), 0.0).astype(np.float32)
    maskT = _tile128(maskT, L)
    cross = (decay ** (L - 1 - i)).astype(np.float32)
    crossb = np.ascontiguousarray(
        np.broadcast_to(np.tile(cross, PCH // L)[None, :], (128, PCH)), np.float32)
    ksc = decay ** np.arange(2 * 128, dtype=np.float64)
    kscale = np.stack([ksc[:128], ksc[128:]], axis=1).astype(np.float32)
    ident = np.eye(128, dtype=np.float32).astype(BF16)

    in_maps = []
    for core in range(N_CORES):
        b, h = divmod(core, 2)
        t0 = h * T_LOC
        te = min(t0 + T_EXT, T)
        xT = np.zeros((V, T_EXT), dtype=BF16)
        xT[:, :te - t0] = x[b, t0:te].T.astype(BF16)
        # pre-tile main: [pch][p][kt][t] contiguous; halo: [p][kt][t]
        xh = np.ascontiguousarray(
            xT[:, :T_LOC].reshape(KT, 128, N_MAIN_PCH, PCH).transpose(2, 1, 0, 3)
        ).reshape(N_MAIN_PCH * 128, KT * PCH)
        xhh = np.ascontiguousarray(
            xT[:, T_LOC:].reshape(KT, 128, HALO).transpose(1, 0, 2)
        ).reshape(128, KT * HALO)
        in_maps.append({
            "xh": xh, "xhh": xhh, "wb": wb, "mt": mt, "owT": owT,
            "maskT": maskT, "crossb": crossb, "kscale": kscale,
            "ident": ident,
        })
    return in_maps


def _ensure_ntff_hook():
    """The agent image's antenv package lacks axon_hooks; shim it so
    run_bass_kernel_spmd(trace=True) can register the NTFF profile hook."""
    try:
        from antenv.axon_hooks import get_axon_ntff_profile_hook  # noqa: F401
        return
    except ImportError:
        pass
    import sys
    import types
    import antenv
    mod = types.ModuleType("antenv.axon_hooks")
    _state = {"hook": None}
    mod.set_axon_ntff_profile_hook = lambda h: _state.__setitem__("hook", h)
    mod.get_axon_ntff_profile_hook = lambda: _state["hook"]
    sys.modules["antenv.axon_hooks"] = mod
    antenv.axon_hooks = mod
    from trn_agent_boot.trn_boot import _ntff_profile_via_ctypes
    mod.set_axon_ntff_profile_hook(
        _ntff_profile_via_ctypes("/opt/axon/libaxon_pjrt.so"))


def run(inputs, trace=False):
    """Returns (out [B,T,V] float32, BassKernelResults)."""
    if trace:
        _ensure_ntff_hook()
    in_maps = _prep_in_maps(inputs)
    nc = _get_nc()
    res = run_bass_kernel_spmd(nc, in_maps, core_ids=list(range(N_CORES)),
                               trace=trace)
    out = np.zeros((B, T, V), np.float32)
    for core in range(N_CORES):
        b, h = divmod(core, 2)
        outT = np.asarray(res.results[core]["outT"]).astype(np.float32)
        out[b, h * T_LOC:(h + 1) * T_LOC] = outT.T
    return out, res


def kernel(**inputs):
    out, _ = run(inputs, trace=False)
    return out


# revision 29
# speedup vs baseline: 1.0438x; 1.0176x over previous
"""Trainium2 Bass kernel for nn_AssociativeMemoryStep (forward-looking retention).

reference semantics:
    q,k,v,o weights = basis @ {q,k,v,o}_coeffs.T          [V, C]
    q/k/v = x @ w                                         [B, T, C]
    scores[t,s] = (q_t . k_s) * decay^(s-t-1) for s>t     (anti-causal)
    retrieved = scores @ v ; out = retrieved @ o_w.T * out_scale

Factored (basis-space) formulation: all four weights share the factor
`basis` [V, 2NB], so with xb = x @ basis [T, 2NB]:
    scores    = (xb @ M) @ xb^T          M  = q_coeffs^T @ k_coeffs  [2NB,2NB]
    out       = (scores_decayed @ xb) @ Wo'
    Wo'       = v_coeffs^T @ o_coeffs @ basis^T * out_scale          [2NB, V]
i.e. retention runs with q := xb@M, k := v := xb. One V-contraction
instead of three (device FLOPs 5.5G -> 3.7G); M and Wo' are tiny
host-precomputed weights (same spirit as the baseline's w = basis@coeffs^T).

Chunkwise-recurrent backward retention, state S_c = sum_{t in chunk c}
decay^t_rel k_t^T v_t (size [2NB,2NB]) from the NEXT chunk only
(decay^256 ~ 4e-6 truncation), L=256 chunks, fully independent.

Sharding: 8 cores = 4 batches x 2 sequence halves of T_loc=2048, each with a
HALO=256 slice of the following positions recomputed locally.

Device layout: xbT ("kT") is [2NB, T_ext]; q~T/qt~T are [2NB, T]; k~ (decay-
scaled) and v copies in [T, 2NB] layout come from PE transposes of xbT.
Projection work for chunk i+1 (transposes + q~) is emitted during chunk i so
the PE never waits on its own PSUM evacuations.
"""

import contextlib

import numpy as np
import ml_dtypes

import concourse.bass as bass
import concourse.mybir as mybir
import concourse.tile as tile
from concourse import bacc
from concourse.bass_utils import run_bass_kernel_spmd

BF16 = ml_dtypes.bfloat16

B, T, V, C = 4, 4096, 1024, 256   # C = 2*N_BASIS = basis channels
N_CORES = 8
T_LOC = 2048          # main positions per core
HALO = 256            # halo positions (state-only; decay^256 ~ 4e-6 truncation)
T_EXT = T_LOC + HALO
L = 256               # retention chunk
PCH = 512             # projection t-chunk
N_MAIN_PCH = T_LOC // PCH  # 4
NCH = T_EXT // L      # 9 L-chunks (8 main + 1 halo)
N_MAIN_CH = T_LOC // L     # 8
KT = V // 128         # 8 v-ktiles
CT = C // 128         # 2 c-tiles

FP32 = mybir.dt.float32
BF = mybir.dt.bfloat16


def build_nc():
    nc = bacc.Bacc("TRN2", target_bir_lowering=False, debug=False,
                   num_devices=N_CORES)

    # all inputs host-pre-tiled to be contiguous per [128, N] DMA block
    xh_d = nc.dram_tensor("xh", [N_MAIN_PCH * 128, KT * PCH], BF, kind="ExternalInput")
    xhh_d = nc.dram_tensor("xhh", [128, KT * HALO], BF, kind="ExternalInput")
    wb_d = nc.dram_tensor("wb", [128, KT * C], BF, kind="ExternalInput")
    mt_d = nc.dram_tensor("mt", [128, CT * C], BF, kind="ExternalInput")
    owT_d = nc.dram_tensor("owT", [128, CT * V], BF, kind="ExternalInput")
    maskT_d = nc.dram_tensor("maskT", [128, CT * L], FP32, kind="ExternalInput")
    crossb_d = nc.dram_tensor("crossb", [128, PCH], FP32, kind="ExternalInput")
    kscale_d = nc.dram_tensor("kscale", [128, 2], FP32, kind="ExternalInput")
    ident_d = nc.dram_tensor("ident", [128, 128], BF, kind="ExternalInput")
    outT_d = nc.dram_tensor("outT", [V, T_LOC], BF, kind="ExternalOutput")

    with tile.TileContext(nc) as tc:
        build_tile(tc, xh_d, xhh_d, wb_d, mt_d, owT_d, maskT_d, crossb_d,
                   kscale_d, ident_d, outT_d)
    nc.compile()
    return nc


def build_tile(tc, xh_d, xhh_d, wb_d, mt_d, owT_d, maskT_d, crossb_d,
               kscale_d, ident_d, outT_d):
    nc = tc.nc

    ctx = contextlib.ExitStack()
    consts = ctx.enter_context(tc.tile_pool(name="consts", bufs=1))
    xpool = ctx.enter_context(tc.tile_pool(name="xpool", bufs=3))
    big = ctx.enter_context(tc.tile_pool(name="big", bufs=1))
    atmp = ctx.enter_context(tc.tile_pool(name="atmp", bufs=4))
    state = ctx.enter_context(tc.tile_pool(name="state", bufs=3))
    ostage = ctx.enter_context(tc.tile_pool(name="ostage", bufs=6))
    psA = ctx.enter_context(tc.tile_pool(name="psA", bufs=2, space="PSUM"))
    psTr = ctx.enter_context(tc.tile_pool(name="psTr", bufs=2, space="PSUM"))
    psB = ctx.enter_context(tc.tile_pool(name="psB", bufs=2, space="PSUM"))
    psO = ctx.enter_context(tc.tile_pool(name="psO", bufs=2, space="PSUM"))

    # ---- constant tiles; DMA order = need order ----
    # wb split into two tiles (kt 0-3 / 4-7) for finer dependency granularity
    wbA_sb = consts.tile([128, 4, C], BF)
    wbB_sb = consts.tile([128, 4, C], BF)
    mt_sb = consts.tile([128, CT, C], BF)
    owT_sb = consts.tile([128, CT * V], BF)
    maskT_sb = consts.tile([128, CT * L], FP32)
    crossb_sb = consts.tile([128, PCH], FP32)
    kscale_sb = consts.tile([128, 2], FP32)
    ident_sb = consts.tile([128, 128], BF)

    def dma_split(out_tile, in_ap, n, engines):
        # split one big contiguous DMA into n pieces so they spread across
        # HWDGE queues (aggregate bandwidth), rotating the issuing engine
        # (each dma_start costs ~650ns serialized on its sequencer)
        if len(out_tile.shape) == 3:
            g = out_tile.shape[1] // n
            b = out_tile.shape[2]
            for i in range(n):
                engines[i % len(engines)].dma_start(
                    out=out_tile[:, i * g:(i + 1) * g, :],
                    in_=in_ap[:, i * g * b:(i + 1) * g * b].rearrange(
                        "p (a b) -> p a b", b=b))
            return
        w = out_tile.shape[-1] // n
        for i in range(n):
            engines[i % len(engines)].dma_start(
                out=out_tile[:, i * w:(i + 1) * w],
                in_=in_ap[:, i * w:(i + 1) * w])

    # ---- persistent activations ----
    kT_sb = big.tile([128, CT, T_EXT], BF)     # xbT: [c-tile, t] main + halo
    qT_sb = big.tile([128, CT, T_LOC], BF)     # q~T = (xb @ M)^T
    qtT_sb = big.tile([128, CT, T_LOC], BF)    # cross-scaled q~T
    ktil_sb = big.tile([128, T_EXT // 128, C], BF)  # decay^t_rel * xb, [t, c]
    v_sb = big.tile([128, T_EXT // 128, C], BF)     # xb in [t, c] layout
    rT_sb = big.tile([128, CT, T_LOC], BF)     # retrieved^T

    # ---- startup DMAs: halo x + basis weights, 128KB pieces, 3 engines ----
    xthA = xpool.tile([128, 4 * HALO], BF, tag="xthA")
    xthB = xpool.tile([128, 4 * HALO], BF, tag="xthB")
    # startup DMA: few big pieces in need-order. The per-queue DMA slots
    # recycle only once earlier transfers' consumers ran, so a short queue
    # reaches the x chunks sooner; the scalar (ACT) queue carries NO loads
    # so psum evacuations are never stuck behind a blocked dma_start.
    nc.sync.dma_start(out=ident_sb, in_=ident_d.ap())   # first: gates warmup
    nc.gpsimd.dma_start(out=kscale_sb, in_=kscale_d.ap())
    nc.sync.dma_start(out=wbA_sb, in_=wb_d.ap()[:, :4 * C].rearrange(
        "p (a b) -> p a b", b=C))
    nc.gpsimd.dma_start(out=xthA, in_=xhh_d.ap()[:, :4 * HALO])
    nc.gpsimd.dma_start(out=wbB_sb, in_=wb_d.ap()[:, 4 * C:].rearrange(
        "p (a b) -> p a b", b=C))
    nc.sync.dma_start(out=xthB, in_=xhh_d.ap()[:, 4 * HALO:])

    # ---- PE clock warmup: the HAM throttle needs ~3.4us of sustained PE
    # activity to release half-rate clocking; burn the initial DMA wait on
    # dummy transposes of ident so the real matmuls start at full clock
    for i in range(16):
        if i % 2 == 0:
            wps = psTr.tile([128, 128], BF, tag="pst", name="warm")
        else:
            wps = psB.tile([128, 128], BF, tag="ps", name="warm")
        nc.tensor.transpose(wps, ident_sb, ident_sb)
    # scratch psum for fillers interleaved with the DMA-paced halo matmuls
    # (slot is recycled by later real allocs; all filler writes precede them)
    warm_ps = psB.tile([128, 128], BF, tag="ps", name="warm_ps")

    # ---- halo xbT (kt-major so each DMA piece gates only its own matmuls) ----
    hps0 = psA.tile([128, HALO], FP32, tag="ps")
    hps1 = psA.tile([128, HALO], FP32, tag="ps")
    hps = [hps0, hps1]
    for kt in range(KT):
        xth = xthA if kt < 4 else xthB
        wbt = wbA_sb if kt < 4 else wbB_sb
        for ct in range(CT):
            nc.tensor.matmul(
                hps[ct], lhsT=wbt[:, kt % 4, ct * 128:(ct + 1) * 128],
                rhs=xth[:, (kt % 4) * HALO:(kt % 4 + 1) * HALO],
                start=(kt == 0), stop=(kt == KT - 1))
        # fillers run whenever the next kt's data hasn't landed yet, keeping
        # the PE activity monitor hot through the DMA-paced halo phase
        nc.tensor.transpose(warm_ps, ident_sb, ident_sb)
        nc.tensor.transpose(warm_ps, ident_sb, ident_sb)
    nc.scalar.copy(kT_sb[:, 0, T_LOC:T_EXT], hps[0])
    nc.vector.tensor_copy(kT_sb[:, 1, T_LOC:T_EXT], hps[1])

    def transposes(t0, nt):
        # k~ and v (both [t, c] layout) via PE transpose of xbT; ct0/ct1 go
        # into one [128, C] psum so the scale/copy evacuations are 256-wide.
        # v copies are emitted later (v_copies) so the ACT queue serves the
        # urgent qT evacuations first.
        psts = []
        for tb in range(nt):
            tt = t0 // 128 + tb
            if tb % 2 == 0:
                pst = psTr.tile([128, C], BF, tag="pst")
            else:
                pst = psB.tile([128, C], BF, tag="ps")
            for ct in range(CT):
                nc.tensor.transpose(
                    pst[:, ct * 128:(ct + 1) * 128],
                    kT_sb[:, ct, t0 + tb * 128:t0 + (tb + 1) * 128], ident_sb)
            nc.vector.tensor_scalar_mul(
                ktil_sb[:, tt, :], pst, kscale_sb[:, (tt % 2):(tt % 2) + 1])
            psts.append((pst, tt))
        return psts

    def v_copies(psts):
        for pst, tt in psts:
            nc.scalar.copy(v_sb[:, tt, :], pst)

    def qproj(pch):
        # q~T = M^T @ xbT (contract over c'). Uses psO (idle until retention)
        # so the slot handoff never blocks the next chunk's xbT matmuls; the
        # psum has a SINGLE reader (Tile serializes same-bank readers), with
        # the cross-scaled copy computed from SBUF later (qtT_muls).
        t0 = pch * PCH
        for ct in range(CT):
            qps = psO.tile([128, PCH], FP32, tag="po", name="qps")
            for cp in range(CT):
                nc.tensor.matmul(
                    qps, lhsT=mt_sb[:, cp, ct * 128:(ct + 1) * 128],
                    rhs=kT_sb[:, cp, t0:t0 + PCH],
                    start=(cp == 0), stop=(cp == CT - 1))
            nc.scalar.copy(qT_sb[:, ct, t0:t0 + PCH], qps)

    def qtT_muls(pch):
        t0 = pch * PCH
        for ct in range(CT):
            nc.vector.tensor_mul(qtT_sb[:, ct, t0:t0 + PCH],
                                 qT_sb[:, ct, t0:t0 + PCH], crossb_sb)

    # ---- main projections, reverse t-chunk order, 1-chunk pipelined ----
    for idx, pch in enumerate(range(N_MAIN_PCH - 1, -1, -1)):
        t0 = pch * PCH
        xt = xpool.tile([128, KT * PCH], BF, tag="xt")
        dma_split(xt, xh_d.ap()[pch * 128:(pch + 1) * 128, :], 4,
                  [nc.sync, nc.gpsimd])
        if idx == 1:
            dma_split(owT_sb, owT_d.ap(), 2, [nc.sync, nc.gpsimd])
            # small consts, needed only once qproj/retention start
            nc.gpsimd.dma_start(out=mt_sb,
                                in_=mt_d.ap().rearrange("p (a b) -> p a b", b=C))
            nc.gpsimd.dma_start(out=crossb_sb, in_=crossb_d.ap())
            nc.gpsimd.dma_start(out=maskT_sb, in_=maskT_d.ap())
        # pipelined: previous chunk's transposes + q~ run while this chunk's
        # x streams in (their inputs were copied out a full chunk ago)
        if idx == 0:
            psts = transposes(T_LOC, HALO // 128)
        else:
            psts = transposes((pch + 1) * PCH, PCH // 128)
            qproj(pch + 1)
        pcs = [psA.tile([128, PCH], FP32, tag="ps", name="xps")
               for _ in range(CT)]
        for kt in range(KT):
            wbt = wbA_sb if kt < 4 else wbB_sb
            for ct in range(CT):
                nc.tensor.matmul(
                    pcs[ct], lhsT=wbt[:, kt % 4, ct * 128:(ct + 1) * 128],
                    rhs=xt[:, kt * PCH:(kt + 1) * PCH],
                    start=(kt == 0), stop=(kt == KT - 1))
        # queue order matters: kT evacs free the psA slots the NEXT chunk's
        # xbT needs, so they go before the deferrable qtT muls / v copies
        # queue order matters: the kT evacs free the psA slots the NEXT
        # chunk's xbT needs, so they precede the deferrable qtT muls/v copies
        nc.scalar.copy(kT_sb[:, 0, t0:t0 + PCH], pcs[0])
        nc.vector.tensor_copy(kT_sb[:, 1, t0:t0 + PCH], pcs[1])
        if idx != 0:
            qtT_muls(pch + 1)
        v_copies(psts)
    psts = transposes(0, PCH // 128)
    qproj(0)
    qtT_muls(0)
    v_copies(psts)

    # ---- phase 2+3 interleaved: backward retention chunks + output proj ----
    S_cur = None

    def retention_chunk(c):
        nonlocal S_cur
        is_main = c < N_MAIN_CH
        c0 = c * L
        tt0 = c0 // 128  # first t-tile of this chunk (2 per chunk)
        atm = []
        rps = []
        if is_main:
            # AT[j, i] = sum_c' k[j,c'] q[i,c'] ; masked -> atm (bf16)
            for jt in range(2):
                ps = psA.tile([128, L], FP32, tag="ps")
                for ct in range(CT):
                    nc.tensor.matmul(
                        ps, lhsT=kT_sb[:, ct, c0 + jt * 128:c0 + (jt + 1) * 128],
                        rhs=qT_sb[:, ct, c0:c0 + L],
                        start=(ct == 0), stop=(ct == CT - 1))
                am = atmp.tile([128, L], BF, tag="atm")
                nc.vector.tensor_mul(am, ps, maskT_sb[:, jt * L:(jt + 1) * L])
                atm.append(am)
            # cross part first: S^T @ q~T needs no masks, so PE keeps running
            # while DVE applies them
            for ct in range(CT):
                ps = psB.tile([128, L], FP32, tag="ps")
                for st in range(CT):
                    nc.tensor.matmul(
                        ps, lhsT=S_cur[:, st, ct * 128:(ct + 1) * 128],
                        rhs=qtT_sb[:, st, c0:c0 + L],
                        start=(st == 0), stop=False)
                rps.append(ps)
        # state: S = k~^T v of THIS chunk only (decay^L * older state ~ 4e-6,
        # numerically negligible -> no recursion, chunks fully independent).
        if c > 0:
            S_new = state.tile([128, CT, C], BF, tag="S")
            for st in range(CT):
                ps = psTr.tile([128, C], FP32, tag="pst")
                for jt in range(2):
                    nc.tensor.matmul(
                        ps, lhsT=ktil_sb[:, tt0 + jt, st * 128:(st + 1) * 128],
                        rhs=v_sb[:, tt0 + jt, :],
                        start=(jt == 0), stop=(jt == 1))
                nc.scalar.copy(S_new[:, st, :], ps)
            S_cur = S_new
        if is_main:
            # intra part: v^T @ atm accumulated into the same rT psum
            for ct in range(CT):
                for jt in range(2):
                    nc.tensor.matmul(
                        rps[ct], lhsT=v_sb[:, tt0 + jt, ct * 128:(ct + 1) * 128],
                        rhs=atm[jt], start=False, stop=(jt == 1))
                nc.vector.tensor_copy(rT_sb[:, ct, c0:c0 + L], rps[ct])

    def outproj(t0, width, pools, final=False):
        # out[v, t0:t0+width]: owT^T @ rT. Output staged 4 v-tiles at a time
        # into one tile -> ONE wide HWDGE store per group (2 per call); no
        # SWDGE stores, so the kernel-end gpsimd DRAIN never waits on them.
        gsz = 4
        ogs = [ostage.tile([128, gsz, width], BF, tag="og", name="og",
                           padded_shape=[128, gsz, PCH])
               for _ in range(KT // gsz)]
        for vt in range(KT):
            pool, tag = pools[vt % len(pools)]
            ps = pool.tile([128, width], FP32, tag=tag, name="ops")
            for ct in range(CT):
                nc.tensor.matmul(
                    ps,
                    lhsT=owT_sb[:, ct * V + vt * 128:ct * V + (vt + 1) * 128],
                    rhs=rT_sb[:, ct, t0:t0 + width],
                    start=(ct == 0), stop=(ct == CT - 1))
            # alternate evac engines so consecutive psums drain concurrently
            cp = nc.vector.tensor_copy if vt % 2 == 0 else nc.scalar.copy
            cp(ogs[vt // gsz][:, vt % gsz, :], ps)
            if vt % gsz == gsz - 1:
                g0 = vt - gsz + 1
                eng = nc.scalar if (final and (vt // gsz) % 2 == 1) else nc.sync
                eng.dma_start(
                    out=outT_d.ap()[g0 * 128:(vt + 1) * 128,
                                    t0:t0 + width].rearrange(
                                        "(a p) t -> p a t", p=128),
                    in_=ogs[vt // gsz])

    # emit outproj one chunk after its rT inputs complete, so the rT
    # evacuations (DVE) have slack before PE consumes them; the last two
    # chunks project at L-granularity so the post-retention tail is short
    for c in range(NCH - 1, -1, -1):
        retention_chunk(c)
        if c in (5, 3):
            outproj((c // 2 + 1) * PCH, PCH, pools=[(psO, "po"), (psB, "ps")])
        elif c == 1:
            outproj(PCH, PCH, pools=[(psO, "po"), (psB, "ps")])
            outproj(L, L, pools=[(psO, "po"), (psB, "ps")])
    outproj(0, L, final=True,
            pools=[(psO, "po"), (psTr, "pst"), (psA, "ps")])

    ctx.close()


# ---------------- host side ----------------

_NC_CACHE = None


def _get_nc():
    global _NC_CACHE
    if _NC_CACHE is None:
        _NC_CACHE = build_nc()
    return _NC_CACHE


def _tile128(a, inner):
    """[G*128, inner] -> [128, G*inner] with block g at columns [g*inner,...)."""
    g = a.shape[0] // 128
    return np.ascontiguousarray(
        a.reshape(g, 128, inner).transpose(1, 0, 2).reshape(128, g * inner))


def _prep_in_maps(inputs):
    x = np.asarray(inputs["x"], np.float32)
    basis = np.asarray(inputs["basis"], np.float32)
    decay = float(1.0 / (1.0 + np.exp(-np.float64(inputs["decay_logit"]))))
    out_scale = float(np.float32(inputs["out_scale"]))

    qc = np.asarray(inputs["q_coeffs"], np.float32)
    kc = np.asarray(inputs["k_coeffs"], np.float32)
    vc = np.asarray(inputs["v_coeffs"], np.float32)
    oc = np.asarray(inputs["o_coeffs"], np.float32)

    wb = _tile128(basis.astype(BF16), C)                       # [128, KT*C]
    mt = _tile128((qc.T @ kc).astype(BF16), C)                 # [128, CT*C]
    owT = _tile128(((vc.T @ oc) @ basis.T * out_scale).astype(BF16), V)

    i = np.arange(L)
    jj, ii = np.meshgrid(i, i, indexing="ij")
    maskT = np.where(jj > ii, decay ** np.maximum(jj - ii - 1, 0), 0.0).astype(np.float32)
    maskT = _tile128(maskT, L)
    cross = (decay ** (L - 1 - i)).astype(np.float32)
    crossb = np.ascontiguousarray(
        np.broadcast_to(np.tile(cross, PCH // L)[None, :], (128, PCH)), np.float32)
    ksc = decay ** np.arange(2 * 128, dtype=np.float64)
    kscale = np.stack([ksc[:128], ksc[128:]], axis=1).astype(np.float32)
    ident = np.eye(128, dtype=np.float32).astype(BF16)

    in_maps = []
    for core in range(N_CORES):
        b, h = divmod(core, 2)
        t0 = h * T_LOC
        te = min(t0 + T_EXT, T)
        xT = np.zeros((V, T_EXT), dtype=BF16)
        xT[:, :te - t0] = x[b, t0:te].T.astype(BF16)
        # pre-tile main: [pch][p][kt][t] contiguous; halo: [p][kt][t]
        xh = np.ascontiguousarray(
            xT[:, :T_LOC].reshape(KT, 128, N_MAIN_PCH, PCH).transpose(2, 1, 0, 3)
        ).reshape(N_MAIN_PCH * 128, KT * PCH)
        xhh = np.ascontiguousarray(
            xT[:, T_LOC:].reshape(KT, 128, HALO).transpose(1, 0, 2)
        ).reshape(128, KT * HALO)
        in_maps.append({
            "xh": xh, "xhh": xhh, "wb": wb, "mt": mt, "owT": owT,
            "maskT": maskT, "crossb": crossb, "kscale": kscale,
            "ident": ident,
        })
    return in_maps


def _ensure_ntff_hook():
    """The agent image's antenv package lacks axon_hooks; shim it so
    run_bass_kernel_spmd(trace=True) can register the NTFF profile hook."""
    try:
        from antenv.axon_hooks import get_axon_ntff_profile_hook  # noqa: F401
        return
    except ImportError:
        pass
    import sys
    import types
    import antenv
    mod = types.ModuleType("antenv.axon_hooks")
    _state = {"hook": None}
    mod.set_axon_ntff_profile_hook = lambda h: _state.__setitem__("hook", h)
    mod.get_axon_ntff_profile_hook = lambda: _state["hook"]
    sys.modules["antenv.axon_hooks"] = mod
    antenv.axon_hooks = mod
    from trn_agent_boot.trn_boot import _ntff_profile_via_ctypes
    mod.set_axon_ntff_profile_hook(
        _ntff_profile_via_ctypes("/opt/axon/libaxon_pjrt.so"))


def run(inputs, trace=False):
    """Returns (out [B,T,V] float32, BassKernelResults)."""
    if trace:
        _ensure_ntff_hook()
    in_maps = _prep_in_maps(inputs)
    nc = _get_nc()
    res = run_bass_kernel_spmd(nc, in_maps, core_ids=list(range(N_CORES)),
                               trace=trace)
    out = np.zeros((B, T, V), np.float32)
    for core in range(N_CORES):
        b, h = divmod(core, 2)
        outT = np.asarray(res.results[core]["outT"]).astype(np.float32)
        out[b, h * T_LOC:(h + 1) * T_LOC] = outT.T
    return out, res


def kernel(**inputs):
    out, _ = run(inputs, trace=False)
    return out
